# revision 2
# baseline (speedup 1.0000x reference)
"""Trainium2 Bass kernel for MultiHeadDirectionalAttention.

Math insight: the "direction bias" (0.3 * dir_w broadcast over keys) is a
per-(batch,head,query) additive constant along the softmax axis, so it cancels
exactly in softmax. The whole direction-scorer path is a mathematical no-op
for both outputs; the kernel computes plain multi-head attention.

Sharding: 8 cores = 2 batches x 4 head-groups (4 heads each).
Per-core layout is fully transposed ([d, S] projections, [k, q] scores) so the
PE contracts over partitions everywhere with no on-device transposes:
  - host passes query/key/value pre-transposed (x^T: [H, S]) per batch
  - Q^T/K^T [64*2, S] per head pair (scale 1/8 folded into Wq on host, exact)
  - scores^T tile = K^T_tile.T @ Q^T  -> PSUM [k=128, q=512]
  - P^T = exp(scores^T) in fp32r (ScalarE)
  - context^T + softmax denominator in one matmul via ones-augmented V
  - attn^T = P^T * bcast(1/denom), DMA'd out; host transposes per head
  - fc_out row-sharded: per-core partial out^T = fo_g.T @ ctx^T, host reduces
All matmuls run in fp32r (~13-bit mantissa, full bf16-rate on PE).
"""

import numpy as np

import concourse.bacc as bacc
import concourse.mybir as mybir
import concourse.tile as tile
from concourse import bass_utils

F32 = mybir.dt.float32
F32R = mybir.dt.float32r
EXP = mybir.ActivationFunctionType.Exp
IDENT = mybir.ActivationFunctionType.Identity

B, S, H = 2, 2048, 1024
NH, HD = 16, 64
HG = 4          # heads per core
D4 = HG * HD    # 256
NCORES = 8
QC = S // 512   # 4 query chunks of 512
KT = S // 128   # 16 key tiles of 128


def build_nc():
    nc = bacc.Bacc("TRN2", target_bir_lowering=False, debug=False)

    xq = nc.dram_tensor("xq", [H, S], F32, kind="ExternalInput")
    xk = nc.dram_tensor("xk", [H, S], F32, kind="ExternalInput")
    xv = nc.dram_tensor("xv", [H, S], F32, kind="ExternalInput")
    wq = nc.dram_tensor("wq", [H, D4], F32, kind="ExternalInput")
    wk = nc.dram_tensor("wk", [H, D4], F32, kind="ExternalInput")
    wv = nc.dram_tensor("wv", [H, D4], F32, kind="ExternalInput")
    bq = nc.dram_tensor("bq", [2, 128], F32, kind="ExternalInput")
    bk = nc.dram_tensor("bk", [2, 128], F32, kind="ExternalInput")
    bv = nc.dram_tensor("bv", [1, D4], F32, kind="ExternalInput")
    fo = nc.dram_tensor("fo", [D4, H], F32, kind="ExternalInput")

    attnT = nc.dram_tensor("attnT", [HG, S, S], F32, kind="ExternalOutput")
    outT = nc.dram_tensor("outT", [H, S], F32, kind="ExternalOutput")

    xq_v = xq.ap().rearrange("(c p) q -> p c q", p=128).bitcast(F32R)
    xk_v = xk.ap().rearrange("(c p) q -> p c q", p=128).bitcast(F32R)
    xv_v = xv.ap().rearrange("(c p) q -> p c q", p=128).bitcast(F32R)
    wq_v = wq.ap().rearrange("(c p) d -> p c d", p=128).bitcast(F32R)
    wk_v = wk.ap().rearrange("(c p) d -> p c d", p=128).bitcast(F32R)
    wv_v = wv.ap().rearrange("(c p) d -> p c d", p=128).bitcast(F32R)
    attnT_v = attnT.ap().rearrange("h (t p) q -> h p t q", p=128)
    outT_v = outT.ap().rearrange("(t p) q -> p t q", p=128)

    with tile.TileContext(nc) as tc:
        with (
            tc.tile_pool(name="pers", bufs=1) as pers,
            tc.tile_pool(name="ps", bufs=2, space="PSUM") as psp,
        ):
            QT = [pers.tile([128, S], F32R, tag=f"qt{p}", name=f"qt{p}") for p in range(2)]
            KTt = [pers.tile([128, S], F32R, tag=f"kt{p}", name=f"kt{p}") for p in range(2)]
            Vaug = pers.tile([128, KT, HG, HD + 1], F32R, tag="vaug")
            ctxT = [pers.tile([64, S], F32R, tag=f"cxt{h}", name=f"cxt{h}") for h in range(HG)]
            fo_sb = [pers.tile([64, H], F32R, tag=f"fo{h}", name=f"fo{h}") for h in range(HG)]
            bq_sb = pers.tile([128, 2], F32, tag="bq")
            bk_sb = pers.tile([128, 2], F32, tag="bk")
            bv_sb = pers.tile([1, D4], F32, tag="bv")
            bv_bc = pers.tile([128, D4], F32, tag="bvbc")

            for h in range(HG):
                nc.sync.dma_start(
                    fo_sb[h][:], fo.ap()[h * 64 : (h + 1) * 64, :].bitcast(F32R)
                )
            nc.sync.dma_start(bq_sb[:], bq.ap().rearrange("c p -> p c"))
            nc.sync.dma_start(bk_sb[:], bk.ap().rearrange("c p -> p c"))
            nc.sync.dma_start(bv_sb[:], bv.ap())
            nc.gpsimd.partition_broadcast(bv_bc[:], bv_sb[:])
            nc.vector.memset(Vaug[:, :, :, HD : HD + 1].bitcast(F32), 1.0)

            # ---------------- projection phase ----------------
            with (
                tc.tile_pool(name="wpool", bufs=1) as wp,
                tc.tile_pool(name="xpool", bufs=3) as xp,
            ):
                wq_sb = wp.tile([128, 8, D4], F32R, tag="wq")
                wk_sb = wp.tile([128, 8, D4], F32R, tag="wk")
                wv_sb = wp.tile([128, 8, D4], F32R, tag="wv")
                nc.sync.dma_start(wq_sb[:], wq_v)
                nc.sync.dma_start(wk_sb[:], wk_v)
                nc.sync.dma_start(wv_sb[:], wv_v)

                for qc in range(QC):
                    qs = slice(qc * 512, (qc + 1) * 512)
                    # Q^T and K^T, one head-pair (128 d-rows) at a time
                    for name, x_v, w_sb, b_sb, dst in (
                        ("q", xq_v, wq_sb, bq_sb, QT),
                        ("k", xk_v, wk_sb, bk_sb, KTt),
                    ):
                        xt = xp.tile([128, 8, 512], F32R, tag="x")
                        nc.sync.dma_start(xt[:], x_v[:, :, qs])
                        for pair in range(2):
                            ps = psp.tile([128, 512], F32, tag="pj")
                            for c in range(8):
                                nc.tensor.matmul(
                                    ps[:],
                                    w_sb[:, c, pair * 128 : (pair + 1) * 128],
                                    xt[:, c, :],
                                    start=(c == 0),
                                    stop=(c == 7),
                                )
                            nc.scalar.activation(
                                dst[pair][:, qs],
                                ps[:],
                                IDENT,
                                bias=b_sb[:, pair : pair + 1],
                            )
                    # V for the 4 k-tiles covered by this chunk
                    xt = xp.tile([128, 8, 512], F32R, tag="x")
                    nc.sync.dma_start(xt[:], xv_v[:, :, qs])
                    for j in range(4):
                        kt = qc * 4 + j
                        ps = psp.tile([128, 512], F32, tag="pj")
                        for c in range(8):
                            nc.tensor.matmul(
                                ps[:, 0:D4],
                                xt[:, c, j * 128 : (j + 1) * 128],
                                wv_sb[:, c, :],
                                start=(c == 0),
                                stop=(c == 7),
                            )
                        nc.vector.tensor_add(
                            Vaug[:, kt, :, 0:HD],
                            ps[:, 0:D4].rearrange("p (h d) -> p h d", d=HD),
                            bv_bc[:].rearrange("p (h d) -> p h d", d=HD),
                        )

            # ---------------- attention main loop ----------------
            with (
                tc.tile_pool(name="ptpool", bufs=2) as ptp,
                tc.tile_pool(name="rpool", bufs=2) as rp,
            ):
                for h in range(HG):
                    pair, half = divmod(h, 2)
                    rows = slice(64 * half, 64 * half + 64)
                    for qc in range(QC):
                        qs = slice(qc * 512, (qc + 1) * 512)
                        pt = ptp.tile([128, KT, 512], F32R, tag="pt")
                        cps = psp.tile([128, 512], F32, tag="cx")
                        for kt2 in range(KT // 2):
                            sps = psp.tile([128, 2, 512], F32, tag="sc")
                            for j in range(2):
                                kt = 2 * kt2 + j
                                nc.tensor.matmul(
                                    sps[:, j, :],
                                    KTt[pair][rows, kt * 128 : (kt + 1) * 128],
                                    QT[pair][rows, qs],
                                    start=True,
                                    stop=True,
                                )
                            nc.scalar.activation(
                                pt[:, 2 * kt2 : 2 * kt2 + 2, :], sps[:], EXP
                            )
                            for j in range(2):
                                kt = 2 * kt2 + j
                                nc.tensor.matmul(
                                    cps[0 : HD + 1, :],
                                    Vaug[:, kt, h, :],
                                    pt[:, kt, :],
                                    start=(kt == 0),
                                    stop=(kt == KT - 1),
                                )
                        recip = rp.tile([1, 512], F32R, tag="rc")
                        with nc.allow_low_precision(reason="f32r softmax"):
                            nc.vector.reciprocal(recip[:], cps[HD : HD + 1, :])
                        rbc = rp.tile([128, 512], F32R, tag="rbc")
                        nc.gpsimd.partition_broadcast(rbc[:], recip[:])
                        # normalized context slice
                        nc.vector.tensor_copy(ctxT[h][:, qs], cps[0:HD, :])
                        nc.vector.tensor_mul(
                            ctxT[h][:, qs], ctxT[h][:, qs], rbc[0:64, :]
                        )
                        # normalize P^T in place, then write out
                        nc.vector.tensor_mul(
                            pt[:],
                            pt[:],
                            rbc[:, None, :].to_broadcast((128, KT, 512)),
                        )
                        nc.sync.dma_start(
                            attnT_v[h, :, :, qs], pt[:].bitcast(F32)
                        )

            # ---------------- fc_out partial ----------------
            with tc.tile_pool(name="outp", bufs=2) as op_:
                for ot in range(H // 128):
                    osb = op_.tile([128, QC, 512], F32, tag="ou")
                    for qc in range(QC):
                        qs = slice(qc * 512, (qc + 1) * 512)
                        fps = psp.tile([128, 512], F32, tag="pj")
                        for h in range(HG):
                            nc.tensor.matmul(
                                fps[:],
                                fo_sb[h][:, ot * 128 : (ot + 1) * 128],
                                ctxT[h][:, qs],
                                start=(h == 0),
                                stop=(h == HG - 1),
                            )
                        nc.vector.tensor_copy(osb[:, qc, :], fps[:])
                    nc.sync.dma_start(
                        outT_v[:, ot, :], osb[:].rearrange("p a b -> p (a b)")
                    )

    nc.compile()
    return nc


_NC_CACHE = None


def kernel(**inputs):
    global _NC_CACHE
    query = np.asarray(inputs["query"], dtype=np.float32)
    key_in = np.asarray(inputs["key_in"], dtype=np.float32)
    value = np.asarray(inputs["value"], dtype=np.float32)
    Wq_w = np.asarray(inputs["Wq_w"], dtype=np.float32)
    Wq_b = np.asarray(inputs["Wq_b"], dtype=np.float32)
    Wk_w = np.asarray(inputs["Wk_w"], dtype=np.float32)
    Wk_b = np.asarray(inputs["Wk_b"], dtype=np.float32)
    Wv_w = np.asarray(inputs["Wv_w"], dtype=np.float32)
    Wv_b = np.asarray(inputs["Wv_b"], dtype=np.float32)
    fo_w = np.asarray(inputs["fo_w"], dtype=np.float32)
    fo_b = np.asarray(inputs["fo_b"], dtype=np.float32)
    # ds1/ds2/direction_signal: softmax-invariant, unused.

    xT = {b: {} for b in range(B)}
    for b in range(B):
        xT[b]["q"] = np.ascontiguousarray(query[b].T)
        xT[b]["k"] = np.ascontiguousarray(key_in[b].T)
        xT[b]["v"] = np.ascontiguousarray(value[b].T)

    in_maps = []
    for core in range(NCORES):
        b, g = divmod(core, 4)
        cols = slice(g * D4, (g + 1) * D4)
        in_maps.append(
            {
                "xq": xT[b]["q"],
                "xk": xT[b]["k"],
                "xv": xT[b]["v"],
                "wq": np.ascontiguousarray(Wq_w[:, cols]) * np.float32(0.125),
                "wk": np.ascontiguousarray(Wk_w[:, cols]),
                "wv": np.ascontiguousarray(Wv_w[:, cols]),
                "bq": (Wq_b[cols] * np.float32(0.125)).reshape(2, 128).copy(),
                "bk": Wk_b[cols].reshape(2, 128).copy(),
                "bv": Wv_b[cols].reshape(1, D4).copy(),
                "fo": np.ascontiguousarray(fo_w[cols, :]),
            }
        )

    if _NC_CACHE is None:
        _NC_CACHE = build_nc()
    nc = _NC_CACHE

    res = bass_utils.run_bass_kernel_spmd(nc, in_maps, core_ids=list(range(NCORES)))

    attention = np.empty((B, NH, S, S), dtype=np.float32)
    output = np.empty((B, S, H), dtype=np.float32)
    acc = {b: np.zeros((H, S), dtype=np.float32) for b in range(B)}
    for core in range(NCORES):
        b, g = divmod(core, 4)
        r = res.results[core]
        at = r["attnT"]
        for i in range(HG):
            attention[b, g * HG + i] = at[i].T
        acc[b] += r["outT"]
    for b in range(B):
        output[b] = acc[b].T + fo_b[None, :]

    return output, attention


# revision 7
# speedup vs baseline: 1.3803x; 1.3803x over previous
"""Trainium2 Bass kernel for MultiHeadDirectionalAttention.

Math insight: the "direction bias" (0.3 * dir_w broadcast over keys) is a
per-(batch,head,query) additive constant along the softmax axis, so it cancels
exactly in softmax. The whole direction-scorer path is a mathematical no-op
for both outputs; the kernel computes plain multi-head attention.

Sharding: 8 cores = 2 batches x 4 head-groups (4 heads each).
Per-core layout is fully transposed ([d, S] projections, [k, q] scores) so the
PE contracts over partitions everywhere with no on-device transposes:
  - host passes query/key/value pre-transposed (x^T: [H, S]) per batch
  - Q^T/K^T [64*2, S] per head pair (scale 1/8 folded into Wq on host, exact)
  - scores^T tile = K^T_tile.T @ Q^T -> PSUM [k=128, q=512]; the two heads of
    a pair sit in partition halves 0-63 / 64-127, so their matmuls target
    different PE row-groups and run concurrently
  - P^T = exp(scores^T) -> fp16 (ScalarE)
  - context^T + softmax denominator in one fp16 matmul via ones-augmented V
  - attn^T = P^T * bcast(1/denom) in fp16, DMA'd out; host transposes/upcasts
  - fc_out row-sharded: per-core partial out^T = fo_g.T @ ctx^T, host reduces
Scores matmuls run in fp32r (~13-bit mantissa at full bf16 PE rate); the
P/V/context side runs in fp16 (also full rate).
"""

import numpy as np

import concourse.bacc as bacc
import concourse.mybir as mybir
import concourse.tile as tile
from concourse import bass_utils

F32 = mybir.dt.float32
F32R = mybir.dt.float32r
F16 = mybir.dt.float16
EXP = mybir.ActivationFunctionType.Exp
IDENT = mybir.ActivationFunctionType.Identity

B, S, H = 2, 2048, 1024
NH, HD = 16, 64
HG = 4          # heads per core
D4 = HG * HD    # 256
NCORES = 8
QC = S // 512   # 4 query chunks of 512
KT = S // 128   # 16 key tiles of 128


def build_nc():
    nc = bacc.Bacc("TRN2", target_bir_lowering=False, debug=False)

    xq = nc.dram_tensor("xq", [H, S], F32, kind="ExternalInput")
    xk = nc.dram_tensor("xk", [H, S], F32, kind="ExternalInput")
    xv = nc.dram_tensor("xv", [H, S], F32, kind="ExternalInput")
    wq = nc.dram_tensor("wq", [H, D4], F32, kind="ExternalInput")
    wk = nc.dram_tensor("wk", [H, D4], F32, kind="ExternalInput")
    wv = nc.dram_tensor("wv", [H, D4], F32, kind="ExternalInput")
    bq = nc.dram_tensor("bq", [2, 128], F32, kind="ExternalInput")
    bk = nc.dram_tensor("bk", [2, 128], F32, kind="ExternalInput")
    bv = nc.dram_tensor("bv", [1, D4], F32, kind="ExternalInput")
    fo = nc.dram_tensor("fo", [D4, H], F16, kind="ExternalInput")

    attnT = nc.dram_tensor("attnT", [HG, S, S], F16, kind="ExternalOutput")
    outT = nc.dram_tensor("outT", [H, S], F32, kind="ExternalOutput")

    xq_v = xq.ap().rearrange("(c p) q -> p c q", p=128).bitcast(F32R)
    xk_v = xk.ap().rearrange("(c p) q -> p c q", p=128).bitcast(F32R)
    xv_v = xv.ap().rearrange("(c p) q -> p c q", p=128).bitcast(F32R)
    wq_v = wq.ap().rearrange("(c p) d -> p c d", p=128).bitcast(F32R)
    wk_v = wk.ap().rearrange("(c p) d -> p c d", p=128).bitcast(F32R)
    wv_v = wv.ap().rearrange("(c p) d -> p c d", p=128).bitcast(F32R)
    attnT_v = attnT.ap().rearrange("h (t p) q -> h p t q", p=128)
    outT_v = outT.ap().rearrange("(t p) q -> p t q", p=128)

    with tile.TileContext(nc) as tc:
        with (
            tc.tile_pool(name="pers", bufs=1) as pers,
            tc.tile_pool(name="wpool", bufs=1) as wp,
            tc.tile_pool(name="xpool", bufs=3) as xp,
            tc.tile_pool(name="qtpool", bufs=4) as qtp,
            tc.tile_pool(name="ctxp", bufs=2) as cxp,
            tc.tile_pool(name="ptpool", bufs=2) as ptp,
            tc.tile_pool(name="rpool", bufs=2) as rp,
            tc.tile_pool(name="outp", bufs=1) as op_,
            tc.tile_pool(name="scps", bufs=2, space="PSUM") as scp,
            tc.tile_pool(name="cxps", bufs=1, space="PSUM") as cxps,
            tc.tile_pool(name="pjps", bufs=2, space="PSUM") as pjp,
        ):
            KTt = [pers.tile([128, S], F16, tag=f"kt{p}", name=f"kt{p}") for p in range(2)]
            Vaug = pers.tile([128, KT, HG, HD + 1], F16, tag="vaug")
            fo_sb = [pers.tile([64, H], F16, tag=f"fo{h}", name=f"fo{h}") for h in range(HG)]
            bq_sb = pers.tile([128, 2], F32, tag="bq")
            bk_sb = pers.tile([128, 2], F32, tag="bk")
            bv_sb = pers.tile([1, D4], F32, tag="bv")
            bv_bc = pers.tile([128, D4], F32, tag="bvbc")

            wq_sb = wp.tile([128, 8, D4], F32R, tag="wq")
            wk_sb = wp.tile([128, 8, D4], F32R, tag="wk")
            wv_sb = wp.tile([128, 8, D4], F32R, tag="wv")
            nc.sync.dma_start(wv_sb[:], wv_v)
            nc.sync.dma_start(bv_sb[:], bv.ap())
            nc.gpsimd.partition_broadcast(bv_bc[:], bv_sb[:])
            nc.vector.memset(Vaug[:, :, :, HD : HD + 1], 1.0)

            # ---- V and K projections over the full sequence ----
            first_loads_done = False
            for qc in range(QC):
                qs = slice(qc * 512, (qc + 1) * 512)
                xt = xp.tile([128, 8, 512], F32R, tag="x", name=f"xv{qc}")
                nc.sync.dma_start(xt[:], xv_v[:, :, qs])
                if not first_loads_done:
                    first_loads_done = True
                    nc.sync.dma_start(wk_sb[:], wk_v)
                    nc.sync.dma_start(wq_sb[:], wq_v)
                    nc.sync.dma_start(bk_sb[:], bk.ap().rearrange("c p -> p c"))
                    nc.sync.dma_start(bq_sb[:], bq.ap().rearrange("c p -> p c"))
                    for h in range(HG):
                        nc.sync.dma_start(fo_sb[h][:], fo.ap()[h * 64 : (h + 1) * 64, :])
                for j in range(4):
                    kt = qc * 4 + j
                    ps = pjp.tile([128, 512], F32, tag="pj", name=f"psv{kt}")
                    for c in range(8):
                        nc.tensor.matmul(
                            ps[:, 0:D4],
                            xt[:, c, j * 128 : (j + 1) * 128],
                            wv_sb[:, c, :],
                            start=(c == 0),
                            stop=(c == 7),
                        )
                    nc.vector.tensor_add(
                        Vaug[:, kt, :, 0:HD],
                        ps[:, 0:D4].rearrange("p (h d) -> p h d", d=HD),
                        bv_bc[:].rearrange("p (h d) -> p h d", d=HD),
                    )
            for qc in range(QC):
                qs = slice(qc * 512, (qc + 1) * 512)
                xt = xp.tile([128, 8, 512], F32R, tag="x", name=f"xk{qc}")
                nc.sync.dma_start(xt[:], xk_v[:, :, qs])
                for pair in range(2):
                    ps = pjp.tile([128, 512], F32, tag="pj", name=f"psk{qc}{pair}")
                    for c in range(8):
                        nc.tensor.matmul(
                            ps[:],
                            wk_sb[:, c, pair * 128 : (pair + 1) * 128],
                            xt[:, c, :],
                            start=(c == 0),
                            stop=(c == 7),
                        )
                    nc.scalar.activation(
                        KTt[pair][:, qs], ps[:], IDENT,
                        bias=bk_sb[:, pair : pair + 1],
                    )

            # ---- per-chunk: Q projection, attention, fc_out ----
            for qc in range(QC):
                qs = slice(qc * 512, (qc + 1) * 512)
                xt = xp.tile([128, 8, 512], F32R, tag="x", name=f"xq{qc}")
                nc.sync.dma_start(xt[:], xq_v[:, :, qs])
                QTc = []
                for pair in range(2):
                    ps = pjp.tile([128, 512], F32, tag="pj", name=f"psq{qc}{pair}")
                    for c in range(8):
                        nc.tensor.matmul(
                            ps[:],
                            wq_sb[:, c, pair * 128 : (pair + 1) * 128],
                            xt[:, c, :],
                            start=(c == 0),
                            stop=(c == 7),
                        )
                    qt = qtp.tile([128, 512], F16, tag="qt", name=f"qt{qc}{pair}")
                    nc.scalar.activation(
                        qt[:], ps[:], IDENT, bias=bq_sb[:, pair : pair + 1]
                    )
                    QTc.append(qt)

                ctx_tiles = {}
                for pair in range(2):
                    pt = ptp.tile([128, KT, 2, 512], F16, tag="pt", name=f"pt{qc}{pair}")
                    cps = [
                        cxps.tile([128, 512], F32, tag=f"cx{hh}", name=f"cx{qc}{pair}{hh}")
                        for hh in range(2)
                    ]
                    for kt in range(KT):
                        ks = slice(kt * 128, (kt + 1) * 128)
                        sps = scp.tile([128, 2, 512], F32, tag="sc", name=f"sc{qc}{pair}{kt}")
                        for hh in range(2):
                            rows = slice(64 * hh, 64 * hh + 64)
                            nc.tensor.matmul(
                                sps[:, hh, :],
                                KTt[pair][rows, ks],
                                QTc[pair][rows, :],
                                start=True,
                                stop=True,
                            )
                        nc.scalar.activation(pt[:, kt, :, :], sps[:], EXP)
                        for hh in range(2):
                            nc.tensor.matmul(
                                cps[hh][0 : HD + 1, :],
                                Vaug[:, kt, 2 * pair + hh, :],
                                pt[:, kt, hh, :],
                                start=(kt == 0),
                                stop=(kt == KT - 1),
                            )
                    for hh in range(2):
                        h = 2 * pair + hh
                        recip = rp.tile([1, 512], F16, tag=f"rc{hh}", name=f"rc{qc}{h}")
                        with nc.allow_low_precision(reason="fp16 softmax"):
                            nc.vector.reciprocal(recip[:], cps[hh][HD : HD + 1, :])
                        rbc = rp.tile([128, 512], F16, tag=f"rbc{hh}", name=f"rbc{qc}{h}")
                        nc.gpsimd.partition_broadcast(rbc[:], recip[:])
                        ctx = cxp.tile([64, 512], F16, tag=f"ctx{h}", name=f"ctx{qc}{h}")
                        nc.vector.tensor_copy(ctx[:], cps[hh][0:HD, :])
                        nc.vector.tensor_mul(ctx[:], ctx[:], rbc[0:64, :])
                        ctx_tiles[h] = ctx
                        nc.vector.tensor_mul(
                            pt[:, :, hh, :],
                            pt[:, :, hh, :],
                            rbc[:, None, :].to_broadcast((128, KT, 512)),
                        )
                        nc.sync.dma_start(attnT_v[h, :, :, qs], pt[:, :, hh, :])

                osb = op_.tile([128, 8, 512], F32, tag="ou", name=f"ou{qc}")
                for ot in range(H // 128):
                    fps = scp.tile([128, 2, 512], F32, tag="sc", name=f"psf{qc}{ot}")[:, 0, :]
                    for h in range(HG):
                        nc.tensor.matmul(
                            fps[:],
                            fo_sb[h][:, ot * 128 : (ot + 1) * 128],
                            ctx_tiles[h][:],
                            start=(h == 0),
                            stop=(h == HG - 1),
                        )
                    nc.vector.tensor_copy(osb[:, ot, :], fps[:])
                nc.sync.dma_start(outT_v[:, :, qs], osb[:])

    nc.compile()
    return nc


_NC_CACHE = None


def kernel(**inputs):
    global _NC_CACHE
    query = np.asarray(inputs["query"], dtype=np.float32)
    key_in = np.asarray(inputs["key_in"], dtype=np.float32)
    value = np.asarray(inputs["value"], dtype=np.float32)
    Wq_w = np.asarray(inputs["Wq_w"], dtype=np.float32)
    Wq_b = np.asarray(inputs["Wq_b"], dtype=np.float32)
    Wk_w = np.asarray(inputs["Wk_w"], dtype=np.float32)
    Wk_b = np.asarray(inputs["Wk_b"], dtype=np.float32)
    Wv_w = np.asarray(inputs["Wv_w"], dtype=np.float32)
    Wv_b = np.asarray(inputs["Wv_b"], dtype=np.float32)
    fo_w = np.asarray(inputs["fo_w"], dtype=np.float32)
    fo_b = np.asarray(inputs["fo_b"], dtype=np.float32)
    # ds1/ds2/direction_signal: softmax-invariant, unused.

    xT = {b: {} for b in range(B)}
    for b in range(B):
        xT[b]["q"] = np.ascontiguousarray(query[b].T)
        xT[b]["k"] = np.ascontiguousarray(key_in[b].T)
        xT[b]["v"] = np.ascontiguousarray(value[b].T)

    in_maps = []
    for core in range(NCORES):
        b, g = divmod(core, 4)
        cols = slice(g * D4, (g + 1) * D4)
        in_maps.append(
            {
                "xq": xT[b]["q"],
                "xk": xT[b]["k"],
                "xv": xT[b]["v"],
                "wq": np.ascontiguousarray(Wq_w[:, cols]) * np.float32(0.125),
                "wk": np.ascontiguousarray(Wk_w[:, cols]),
                "wv": np.ascontiguousarray(Wv_w[:, cols]),
                "bq": (Wq_b[cols] * np.float32(0.125)).reshape(2, 128).copy(),
                "bk": Wk_b[cols].reshape(2, 128).copy(),
                "bv": Wv_b[cols].reshape(1, D4).copy(),
                "fo": np.ascontiguousarray(fo_w[cols, :]).astype(np.float16),
            }
        )

    if _NC_CACHE is None:
        _NC_CACHE = build_nc()
    nc = _NC_CACHE

    res = bass_utils.run_bass_kernel_spmd(nc, in_maps, core_ids=list(range(NCORES)))

    attention = np.empty((B, NH, S, S), dtype=np.float32)
    output = np.empty((B, S, H), dtype=np.float32)
    acc = {b: np.zeros((H, S), dtype=np.float32) for b in range(B)}
    for core in range(NCORES):
        b, g = divmod(core, 4)
        r = res.results[core]
        at = r["attnT"]
        for i in range(HG):
            attention[b, g * HG + i] = at[i].T
        acc[b] += r["outT"]
    for b in range(B):
        output[b] = acc[b].T + fo_b[None, :]

    return output, attention


# revision 11
# speedup vs baseline: 1.5217x; 1.1024x over previous
"""Trainium2 Bass kernel for MultiHeadDirectionalAttention.

Math insight: the "direction bias" (0.3 * dir_w broadcast over keys) is a
per-(batch,head,query) additive constant along the softmax axis, so it cancels
exactly in softmax. The whole direction-scorer path is a mathematical no-op
for both outputs; the kernel computes plain multi-head attention.

Sharding: 8 cores = 2 batches x 4 head-groups (4 heads each).
Per-core layout is fully transposed ([d, S] projections, [k, q] scores) so the
PE contracts over partitions everywhere with no on-device transposes:
  - host passes query/key/value pre-transposed (x^T: [H, S]) per batch
  - Q^T/K^T [64*2, S] per head pair (scale 1/8 folded into Wq on host, exact)
  - scores^T tile = K^T_tile.T @ Q^T -> PSUM [k=128, q=512]; the two heads of
    a pair sit in partition halves 0-63 / 64-127, so their matmuls target
    different PE row-groups and run concurrently
  - P^T = exp(scores^T) -> fp16 (ScalarE)
  - context^T + softmax denominator in one fp16 matmul via ones-augmented V
  - attn^T = P^T * bcast(1/denom) in fp16, DMA'd out; host transposes/upcasts
  - fc_out row-sharded: per-core partial out^T = fo_g.T @ ctx^T, host reduces
Scores matmuls run in fp32r (~13-bit mantissa at full bf16 PE rate); the
P/V/context side runs in fp16 (also full rate).
"""

import numpy as np

import concourse.bacc as bacc
import concourse.mybir as mybir
import concourse.tile as tile
from concourse import bass_utils

F32 = mybir.dt.float32
F32R = mybir.dt.float32r
F16 = mybir.dt.float16
EXP = mybir.ActivationFunctionType.Exp
IDENT = mybir.ActivationFunctionType.Identity

B, S, H = 2, 2048, 1024
NH, HD = 16, 64
HG = 4          # heads per core
D4 = HG * HD    # 256
NCORES = 8
QC = S // 512   # 4 query chunks of 512
KT = S // 128   # 16 key tiles of 128


def build_nc():
    nc = bacc.Bacc("TRN2", target_bir_lowering=False, debug=False)

    xq = nc.dram_tensor("xq", [H, S], F16, kind="ExternalInput")
    xk = nc.dram_tensor("xk", [H, S], F16, kind="ExternalInput")
    xv = nc.dram_tensor("xv", [H, S], F16, kind="ExternalInput")
    wq = nc.dram_tensor("wq", [H, D4], F16, kind="ExternalInput")
    wk = nc.dram_tensor("wk", [H, D4], F16, kind="ExternalInput")
    wv = nc.dram_tensor("wv", [H, D4], F16, kind="ExternalInput")
    bq = nc.dram_tensor("bq", [2, 128], F32, kind="ExternalInput")
    bk = nc.dram_tensor("bk", [2, 128], F32, kind="ExternalInput")
    bv = nc.dram_tensor("bv", [1, D4], F32, kind="ExternalInput")
    fo = nc.dram_tensor("fo", [D4, H], F16, kind="ExternalInput")

    attnT = nc.dram_tensor("attnT", [HG, S, S], F16, kind="ExternalOutput")
    outT = nc.dram_tensor("outT", [H, S], F16, kind="ExternalOutput")

    xq_v = xq.ap().rearrange("(c p) q -> p c q", p=128)
    xk_v = xk.ap().rearrange("(c p) q -> p c q", p=128)
    xv_v = xv.ap().rearrange("(c p) q -> p c q", p=128)
    wq_v = wq.ap().rearrange("(c p) d -> p c d", p=128)
    wk_v = wk.ap().rearrange("(c p) d -> p c d", p=128)
    wv_v = wv.ap().rearrange("(c p) d -> p c d", p=128)
    attnT_v = attnT.ap().rearrange("h (t p) q -> h p t q", p=128)
    outT_v = outT.ap().rearrange("(t p) q -> p t q", p=128)

    with tile.TileContext(nc) as tc:
        with (
            tc.tile_pool(name="pers", bufs=1) as pers,
            tc.tile_pool(name="wpool", bufs=1) as wp,
            tc.tile_pool(name="xpool", bufs=3) as xp,
            tc.tile_pool(name="qtpool", bufs=4) as qtp,
            tc.tile_pool(name="ctxp", bufs=2) as cxp,
            tc.tile_pool(name="ptpool", bufs=2) as ptp,
            tc.tile_pool(name="rpool", bufs=2) as rp,
            tc.tile_pool(name="outp", bufs=1) as op_,
            tc.tile_pool(name="scps", bufs=2, space="PSUM") as scp,
            tc.tile_pool(name="cxps", bufs=1, space="PSUM") as cxps,
            tc.tile_pool(name="pjps", bufs=2, space="PSUM") as pjp,
        ):
            KTt = [pers.tile([128, S], F16, tag=f"kt{p}", name=f"kt{p}") for p in range(2)]
            Vaug = pers.tile([128, KT, HG, HD + 1], F16, tag="vaug")
            fo_sb = [pers.tile([64, H], F16, tag=f"fo{h}", name=f"fo{h}") for h in range(HG)]
            bq_sb = pers.tile([128, 2], F32, tag="bq")
            bk_sb = pers.tile([128, 2], F32, tag="bk")
            bv_sb = pers.tile([1, D4], F32, tag="bv")
            bv_bc = pers.tile([128, D4], F32, tag="bvbc")

            wq_sb = wp.tile([128, 8, D4], F16, tag="wq")
            wk_sb = wp.tile([128, 8, D4], F16, tag="wk")
            wv_sb = wp.tile([128, 8, D4], F16, tag="wv")
            nc.sync.dma_start(wv_sb[:], wv_v)
            nc.sync.dma_start(bv_sb[:], bv.ap())
            nc.gpsimd.partition_broadcast(bv_bc[:], bv_sb[:])
            nc.vector.memset(Vaug[:, :, :, HD : HD + 1], 1.0)

            # ---- V and K projections over the full sequence ----
            first_loads_done = False
            for qc in range(QC):
                qs = slice(qc * 512, (qc + 1) * 512)
                xt = xp.tile([128, 8, 512], F16, tag="x", name=f"xv{qc}")
                nc.sync.dma_start(xt[:], xv_v[:, :, qs])
                if not first_loads_done:
                    first_loads_done = True
                    nc.sync.dma_start(wk_sb[:], wk_v)
                    nc.sync.dma_start(wq_sb[:], wq_v)
                    nc.sync.dma_start(bk_sb[:], bk.ap().rearrange("c p -> p c"))
                    nc.sync.dma_start(bq_sb[:], bq.ap().rearrange("c p -> p c"))
                    for h in range(HG):
                        nc.sync.dma_start(fo_sb[h][:], fo.ap()[h * 64 : (h + 1) * 64, :])
                for j in range(4):
                    kt = qc * 4 + j
                    ps = pjp.tile([128, 512], F32, tag="pj", name=f"psv{kt}")
                    for c in range(8):
                        nc.tensor.matmul(
                            ps[:, 0:D4],
                            xt[:, c, j * 128 : (j + 1) * 128],
                            wv_sb[:, c, :],
                            start=(c == 0),
                            stop=(c == 7),
                        )
                    nc.vector.tensor_add(
                        Vaug[:, kt, :, 0:HD],
                        ps[:, 0:D4].rearrange("p (h d) -> p h d", d=HD),
                        bv_bc[:].rearrange("p (h d) -> p h d", d=HD),
                    )
            for qc in range(QC):
                qs = slice(qc * 512, (qc + 1) * 512)
                xt = xp.tile([128, 8, 512], F16, tag="x", name=f"xk{qc}")
                nc.sync.dma_start(xt[:], xk_v[:, :, qs])
                for pair in range(2):
                    ps = pjp.tile([128, 512], F32, tag="pj", name=f"psk{qc}{pair}")
                    for c in range(8):
                        nc.tensor.matmul(
                            ps[:],
                            wk_sb[:, c, pair * 128 : (pair + 1) * 128],
                            xt[:, c, :],
                            start=(c == 0),
                            stop=(c == 7),
                        )
                    nc.vector.tensor_scalar_add(
                        KTt[pair][:, qs], ps[:], bk_sb[:, pair : pair + 1]
                    )

            # ---- per-chunk: Q projection, attention, fc_out ----
            for qc in range(QC):
                qs = slice(qc * 512, (qc + 1) * 512)
                xt = xp.tile([128, 8, 512], F16, tag="x", name=f"xq{qc}")
                nc.sync.dma_start(xt[:], xq_v[:, :, qs])
                QTc = []
                for pair in range(2):
                    ps = pjp.tile([128, 512], F32, tag="pj", name=f"psq{qc}{pair}")
                    for c in range(8):
                        nc.tensor.matmul(
                            ps[:],
                            wq_sb[:, c, pair * 128 : (pair + 1) * 128],
                            xt[:, c, :],
                            start=(c == 0),
                            stop=(c == 7),
                        )
                    qt = qtp.tile([128, 512], F16, tag="qt", name=f"qt{qc}{pair}")
                    nc.vector.tensor_scalar_add(qt[:], ps[:], bq_sb[:, pair : pair + 1])
                    QTc.append(qt)

                ctx_tiles = {}
                for pair in range(2):
                    pt = ptp.tile([128, KT, 2, 512], F16, tag="pt", name=f"pt{qc}{pair}")
                    cps = [
                        cxps.tile([128, 512], F32, tag=f"cx{hh}", name=f"cx{qc}{pair}{hh}")
                        for hh in range(2)
                    ]
                    for kt in range(KT):
                        ks = slice(kt * 128, (kt + 1) * 128)
                        sps = scp.tile([128, 2, 512], F32, tag="sc", name=f"sc{qc}{pair}{kt}")
                        for hh in range(2):
                            rows = slice(64 * hh, 64 * hh + 64)
                            nc.tensor.matmul(
                                sps[:, hh, :],
                                KTt[pair][rows, ks],
                                QTc[pair][rows, :],
                                start=True,
                                stop=True,
                            )
                        nc.scalar.activation(pt[:, kt, :, :], sps[:], EXP)
                        for hh in range(2):
                            nc.tensor.matmul(
                                cps[hh][0 : HD + 1, :],
                                Vaug[:, kt, 2 * pair + hh, :],
                                pt[:, kt, hh, :],
                                start=(kt == 0),
                                stop=(kt == KT - 1),
                            )
                    for hh in range(2):
                        h = 2 * pair + hh
                        recip = rp.tile([1, 512], F16, tag=f"rc{hh}", name=f"rc{qc}{h}")
                        with nc.allow_low_precision(reason="fp16 softmax"):
                            nc.vector.reciprocal(recip[:], cps[hh][HD : HD + 1, :])
                        rbc = rp.tile([128, 512], F16, tag=f"rbc{hh}", name=f"rbc{qc}{h}")
                        nc.gpsimd.partition_broadcast(rbc[:], recip[:])
                        ctx = cxp.tile([64, 512], F16, tag=f"ctx{h}", name=f"ctx{qc}{h}")
                        nc.vector.tensor_copy(ctx[:], cps[hh][0:HD, :])
                        nc.vector.tensor_mul(ctx[:], ctx[:], rbc[0:64, :])
                        ctx_tiles[h] = ctx
                        nc.vector.tensor_mul(
                            pt[:, :, hh, :],
                            pt[:, :, hh, :],
                            rbc[:, None, :].to_broadcast((128, KT, 512)),
                        )
                        nc.sync.dma_start(attnT_v[h, :, :, qs], pt[:, :, hh, :])

                osb = op_.tile([128, 8, 512], F16, tag="ou", name=f"ou{qc}")
                for ot in range(H // 128):
                    fps = scp.tile([128, 2, 512], F32, tag="sc", name=f"psf{qc}{ot}")[:, 0, :]
                    for h in range(HG):
                        nc.tensor.matmul(
                            fps[:],
                            fo_sb[h][:, ot * 128 : (ot + 1) * 128],
                            ctx_tiles[h][:],
                            start=(h == 0),
                            stop=(h == HG - 1),
                        )
                    nc.vector.tensor_copy(osb[:, ot, :], fps[:])
                nc.sync.dma_start(outT_v[:, :, qs], osb[:])

    nc.compile()
    return nc


_NC_CACHE = None


def kernel(**inputs):
    global _NC_CACHE
    query = np.asarray(inputs["query"], dtype=np.float32)
    key_in = np.asarray(inputs["key_in"], dtype=np.float32)
    value = np.asarray(inputs["value"], dtype=np.float32)
    Wq_w = np.asarray(inputs["Wq_w"], dtype=np.float32)
    Wq_b = np.asarray(inputs["Wq_b"], dtype=np.float32)
    Wk_w = np.asarray(inputs["Wk_w"], dtype=np.float32)
    Wk_b = np.asarray(inputs["Wk_b"], dtype=np.float32)
    Wv_w = np.asarray(inputs["Wv_w"], dtype=np.float32)
    Wv_b = np.asarray(inputs["Wv_b"], dtype=np.float32)
    fo_w = np.asarray(inputs["fo_w"], dtype=np.float32)
    fo_b = np.asarray(inputs["fo_b"], dtype=np.float32)
    # ds1/ds2/direction_signal: softmax-invariant, unused.

    xT = {b: {} for b in range(B)}
    for b in range(B):
        xT[b]["q"] = query[b].T.astype(np.float16)
        xT[b]["k"] = key_in[b].T.astype(np.float16)
        xT[b]["v"] = value[b].T.astype(np.float16)

    in_maps = []
    for core in range(NCORES):
        b, g = divmod(core, 4)
        cols = slice(g * D4, (g + 1) * D4)
        in_maps.append(
            {
                "xq": xT[b]["q"],
                "xk": xT[b]["k"],
                "xv": xT[b]["v"],
                "wq": (np.ascontiguousarray(Wq_w[:, cols]) * np.float32(0.125)).astype(np.float16),
                "wk": np.ascontiguousarray(Wk_w[:, cols]).astype(np.float16),
                "wv": np.ascontiguousarray(Wv_w[:, cols]).astype(np.float16),
                "bq": (Wq_b[cols] * np.float32(0.125)).reshape(2, 128).copy(),
                "bk": Wk_b[cols].reshape(2, 128).copy(),
                "bv": Wv_b[cols].reshape(1, D4).copy(),
                "fo": np.ascontiguousarray(fo_w[cols, :]).astype(np.float16),
            }
        )

    if _NC_CACHE is None:
        _NC_CACHE = build_nc()
    nc = _NC_CACHE

    res = bass_utils.run_bass_kernel_spmd(nc, in_maps, core_ids=list(range(NCORES)))

    attention = np.empty((B, NH, S, S), dtype=np.float32)
    output = np.empty((B, S, H), dtype=np.float32)
    acc = {b: np.zeros((H, S), dtype=np.float32) for b in range(B)}
    for core in range(NCORES):
        b, g = divmod(core, 4)
        r = res.results[core]
        at = r["attnT"]
        for i in range(HG):
            attention[b, g * HG + i] = at[i].T
        acc[b] += r["outT"].astype(np.float32)
    for b in range(B):
        output[b] = acc[b].T + fo_b[None, :]

    return output, attention


# revision 14
# speedup vs baseline: 1.7725x; 1.1648x over previous
"""Trainium2 Bass kernel for MultiHeadDirectionalAttention.

Math insight: the "direction bias" (0.3 * dir_w broadcast over keys) is a
per-(batch,head,query) additive constant along the softmax axis, so it cancels
exactly in softmax. The whole direction-scorer path is a mathematical no-op
for both outputs; the kernel computes plain multi-head attention.

Sharding: 8 cores = 2 batches x 4 head-groups (4 heads each).
Per-core layout is fully transposed ([d, S] projections, [k, q] scores) so the
PE contracts over partitions everywhere with no on-device transposes:
  - host passes query/key/value pre-transposed (x^T: [H, S]) per batch
  - Q^T/K^T [64*2, S] per head pair (scale 1/8 folded into Wq on host, exact)
  - scores^T tile = K^T_tile.T @ Q^T -> PSUM [k=128, q=512]; the two heads of
    a pair sit in partition halves 0-63 / 64-127, so their matmuls target
    different PE row-groups and run concurrently
  - P^T = exp(scores^T) -> fp16 (ScalarE)
  - context^T + softmax denominator in one fp16 matmul via ones-augmented V
  - attn^T = P^T * bcast(1/denom) in fp16, DMA'd out; host transposes/upcasts
  - fc_out row-sharded: per-core partial out^T = fo_g.T @ ctx^T, host reduces
Scores matmuls run in fp32r (~13-bit mantissa at full bf16 PE rate); the
P/V/context side runs in fp16 (also full rate).
"""

import numpy as np

import concourse.bacc as bacc
import concourse.mybir as mybir
import concourse.tile as tile
from concourse import bass_utils

F32 = mybir.dt.float32
F32R = mybir.dt.float32r
F16 = mybir.dt.float16
EXP = mybir.ActivationFunctionType.Exp
IDENT = mybir.ActivationFunctionType.Identity

B, S, H = 2, 2048, 1024
NH, HD = 16, 64
HG = 4          # heads per core
D4 = HG * HD    # 256
NCORES = 8
QC = S // 512   # 4 query chunks of 512
KT = S // 128   # 16 key tiles of 128


def build_nc():
    nc = bacc.Bacc("TRN2", target_bir_lowering=False, debug=False)

    xq = nc.dram_tensor("xq", [H, S], F16, kind="ExternalInput")
    xk = nc.dram_tensor("xk", [H, S], F16, kind="ExternalInput")
    xv = nc.dram_tensor("xv", [H, S], F16, kind="ExternalInput")
    wq = nc.dram_tensor("wq", [H, D4], F16, kind="ExternalInput")
    wk = nc.dram_tensor("wk", [H, D4], F16, kind="ExternalInput")
    wv = nc.dram_tensor("wv", [H, D4], F16, kind="ExternalInput")
    bq = nc.dram_tensor("bq", [2, 128], F32, kind="ExternalInput")
    bk = nc.dram_tensor("bk", [2, 128], F32, kind="ExternalInput")
    bv = nc.dram_tensor("bv", [1, D4], F32, kind="ExternalInput")
    fo = nc.dram_tensor("fo", [D4, H], F16, kind="ExternalInput")

    attnT = nc.dram_tensor("attnT", [HG, S, S], F16, kind="ExternalOutput")
    outT = nc.dram_tensor("outT", [H, S], F16, kind="ExternalOutput")

    xq_v = xq.ap().rearrange("(c p) q -> p c q", p=128)
    xk_v = xk.ap().rearrange("(c p) q -> p c q", p=128)
    xv_v = xv.ap().rearrange("(c p) q -> p c q", p=128)
    wq_v = wq.ap().rearrange("(c p) d -> p c d", p=128)
    wk_v = wk.ap().rearrange("(c p) d -> p c d", p=128)
    wv_v = wv.ap().rearrange("(c p) d -> p c d", p=128)
    attnT_v = attnT.ap().rearrange("h (t p) q -> h p t q", p=128)
    outT_v = outT.ap().rearrange("(t p) q -> p t q", p=128)

    with tile.TileContext(nc) as tc:
        with (
            tc.tile_pool(name="pers", bufs=1) as pers,
            tc.tile_pool(name="wpool", bufs=1) as wp,
            tc.tile_pool(name="xpool", bufs=3) as xp,
            tc.tile_pool(name="qtpool", bufs=4) as qtp,
            tc.tile_pool(name="ctxp", bufs=2) as cxp,
            tc.tile_pool(name="ptpool", bufs=3) as ptp,
            tc.tile_pool(name="rpool", bufs=2) as rp,
            tc.tile_pool(name="outp", bufs=1) as op_,
            tc.tile_pool(name="scps", bufs=2, space="PSUM") as scp,
            tc.tile_pool(name="cxps", bufs=1, space="PSUM") as cxps,
            tc.tile_pool(name="pjps", bufs=2, space="PSUM") as pjp,
        ):
            KTt = [pers.tile([128, S], F16, tag=f"kt{p}", name=f"kt{p}") for p in range(2)]
            Vaug = pers.tile([128, KT, HG, HD + 1], F16, tag="vaug")
            fo_sb = [pers.tile([64, H], F16, tag=f"fo{h}", name=f"fo{h}") for h in range(HG)]
            bq_sb = pers.tile([128, 2], F32, tag="bq")
            bk_sb = pers.tile([128, 2], F32, tag="bk")
            bv_sb = pers.tile([1, D4], F32, tag="bv")
            bv_bc = pers.tile([128, D4], F32, tag="bvbc")

            wq_sb = wp.tile([128, 8, D4], F16, tag="wq")
            wk_sb = wp.tile([128, 8, D4], F16, tag="wk")
            wv_sb = wp.tile([128, 8, D4], F16, tag="wv")
            nc.sync.dma_start(wv_sb[:], wv_v)
            nc.sync.dma_start(bv_sb[:], bv.ap())
            nc.gpsimd.partition_broadcast(bv_bc[:], bv_sb[:])
            nc.vector.memset(Vaug[:, :, :, HD : HD + 1], 1.0)

            # ---- V and K projections over the full sequence ----
            first_loads_done = False
            for qc in range(QC):
                qs = slice(qc * 512, (qc + 1) * 512)
                xt = xp.tile([128, 8, 512], F16, tag="x", name=f"xv{qc}")
                nc.sync.dma_start(xt[:], xv_v[:, :, qs])
                if not first_loads_done:
                    first_loads_done = True
                    nc.sync.dma_start(wk_sb[:], wk_v)
                    nc.sync.dma_start(wq_sb[:], wq_v)
                    nc.sync.dma_start(bk_sb[:], bk.ap().rearrange("c p -> p c"))
                    nc.sync.dma_start(bq_sb[:], bq.ap().rearrange("c p -> p c"))
                    for h in range(HG):
                        nc.sync.dma_start(fo_sb[h][:], fo.ap()[h * 64 : (h + 1) * 64, :])
                for j in range(4):
                    kt = qc * 4 + j
                    ps = pjp.tile([128, 512], F32, tag="pj", name=f"psv{kt}")
                    for c in range(8):
                        nc.tensor.matmul(
                            ps[:, 0:D4],
                            xt[:, c, j * 128 : (j + 1) * 128],
                            wv_sb[:, c, :],
                            start=(c == 0),
                            stop=(c == 7),
                        )
                    nc.vector.tensor_add(
                        Vaug[:, kt, :, 0:HD],
                        ps[:, 0:D4].rearrange("p (h d) -> p h d", d=HD),
                        bv_bc[:].rearrange("p (h d) -> p h d", d=HD),
                    )
            for qc in range(QC):
                qs = slice(qc * 512, (qc + 1) * 512)
                xt = xp.tile([128, 8, 512], F16, tag="x", name=f"xk{qc}")
                nc.sync.dma_start(xt[:], xk_v[:, :, qs])
                for pair in range(2):
                    ps = pjp.tile([128, 512], F32, tag="pj", name=f"psk{qc}{pair}")
                    for c in range(8):
                        nc.tensor.matmul(
                            ps[:],
                            wk_sb[:, c, pair * 128 : (pair + 1) * 128],
                            xt[:, c, :],
                            start=(c == 0),
                            stop=(c == 7),
                        )
                    nc.vector.tensor_scalar_add(
                        KTt[pair][:, qs], ps[:], bk_sb[:, pair : pair + 1]
                    )

            # ---- per-chunk: attention + fc_out, Q proj pipelined 1 ahead ----
            def q_proj(qc):
                qs = slice(qc * 512, (qc + 1) * 512)
                xt = xp.tile([128, 8, 512], F16, tag="x", name=f"xq{qc}")
                nc.sync.dma_start(xt[:], xq_v[:, :, qs])
                out = []
                for pair in range(2):
                    ps = pjp.tile([128, 512], F32, tag="pj", name=f"psq{qc}{pair}")
                    for c in range(8):
                        nc.tensor.matmul(
                            ps[:],
                            wq_sb[:, c, pair * 128 : (pair + 1) * 128],
                            xt[:, c, :],
                            start=(c == 0),
                            stop=(c == 7),
                        )
                    qt = qtp.tile([128, 512], F16, tag="qt", name=f"qt{qc}{pair}")
                    nc.vector.tensor_scalar_add(qt[:], ps[:], bq_sb[:, pair : pair + 1])
                    out.append(qt)
                return out

            QT_next = q_proj(0)
            for qc in range(QC):
                qs = slice(qc * 512, (qc + 1) * 512)
                QTc = QT_next
                if qc + 1 < QC:
                    QT_next = q_proj(qc + 1)

                ctx_tiles = {}
                for pair in range(2):
                    pt = ptp.tile([128, KT, 2, 512], F16, tag="pt", name=f"pt{qc}{pair}")
                    cps = [
                        cxps.tile([128, 512], F32, tag=f"cx{hh}", name=f"cx{qc}{pair}{hh}")
                        for hh in range(2)
                    ]
                    for kt in range(KT):
                        ks = slice(kt * 128, (kt + 1) * 128)
                        sps = scp.tile([128, 2, 512], F32, tag="sc", name=f"sc{qc}{pair}{kt}")
                        for hh in range(2):
                            rows = slice(64 * hh, 64 * hh + 64)
                            nc.tensor.matmul(
                                sps[:, hh, :],
                                KTt[pair][rows, ks],
                                QTc[pair][rows, :],
                                start=True,
                                stop=True,
                            )
                        nc.scalar.activation(pt[:, kt, :, :], sps[:], EXP)
                        for hh in range(2):
                            nc.tensor.matmul(
                                cps[hh][0 : HD + 1, :],
                                Vaug[:, kt, 2 * pair + hh, :],
                                pt[:, kt, hh, :],
                                start=(kt == 0),
                                stop=(kt == KT - 1),
                            )
                    for hh in range(2):
                        h = 2 * pair + hh
                        recip = rp.tile([1, 512], F16, tag=f"rc{hh}", name=f"rc{qc}{h}")
                        with nc.allow_low_precision(reason="fp16 softmax"):
                            nc.vector.reciprocal(recip[:], cps[hh][HD : HD + 1, :])
                        rbc = rp.tile([128, 512], F16, tag=f"rbc{hh}", name=f"rbc{qc}{h}")
                        nc.gpsimd.partition_broadcast(rbc[:], recip[:])
                        ctx = cxp.tile([64, 512], F16, tag=f"ctx{h}", name=f"ctx{qc}{h}")
                        nc.vector.tensor_copy(ctx[:], cps[hh][0:HD, :])
                        nc.vector.tensor_mul(ctx[:], ctx[:], rbc[0:64, :])
                        ctx_tiles[h] = ctx
                        for half in range(2):
                            kslice = slice(half * (KT // 2), (half + 1) * (KT // 2))
                            nc.vector.tensor_mul(
                                pt[:, kslice, hh, :],
                                pt[:, kslice, hh, :],
                                rbc[:, None, :].to_broadcast((128, KT // 2, 512)),
                            )
                            nc.sync.dma_start(
                                attnT_v[h, :, kslice, qs], pt[:, kslice, hh, :]
                            )

                osb = op_.tile([128, 8, 512], F16, tag="ou", name=f"ou{qc}")
                for ot in range(H // 128):
                    fps = pjp.tile([128, 512], F32, tag="pj", name=f"psf{qc}{ot}")
                    for h in range(HG):
                        nc.tensor.matmul(
                            fps[:],
                            fo_sb[h][:, ot * 128 : (ot + 1) * 128],
                            ctx_tiles[h][:],
                            start=(h == 0),
                            stop=(h == HG - 1),
                        )
                    nc.vector.tensor_copy(osb[:, ot, :], fps[:])
                nc.sync.dma_start(outT_v[:, :, qs], osb[:])

    nc.compile()
    return nc


_NC_CACHE = None


def kernel(**inputs):
    global _NC_CACHE
    query = np.asarray(inputs["query"], dtype=np.float32)
    key_in = np.asarray(inputs["key_in"], dtype=np.float32)
    value = np.asarray(inputs["value"], dtype=np.float32)
    Wq_w = np.asarray(inputs["Wq_w"], dtype=np.float32)
    Wq_b = np.asarray(inputs["Wq_b"], dtype=np.float32)
    Wk_w = np.asarray(inputs["Wk_w"], dtype=np.float32)
    Wk_b = np.asarray(inputs["Wk_b"], dtype=np.float32)
    Wv_w = np.asarray(inputs["Wv_w"], dtype=np.float32)
    Wv_b = np.asarray(inputs["Wv_b"], dtype=np.float32)
    fo_w = np.asarray(inputs["fo_w"], dtype=np.float32)
    fo_b = np.asarray(inputs["fo_b"], dtype=np.float32)
    # ds1/ds2/direction_signal: softmax-invariant, unused.

    xT = {b: {} for b in range(B)}
    for b in range(B):
        xT[b]["q"] = query[b].T.astype(np.float16)
        xT[b]["k"] = key_in[b].T.astype(np.float16)
        xT[b]["v"] = value[b].T.astype(np.float16)

    in_maps = []
    for core in range(NCORES):
        b, g = divmod(core, 4)
        cols = slice(g * D4, (g + 1) * D4)
        in_maps.append(
            {
                "xq": xT[b]["q"],
                "xk": xT[b]["k"],
                "xv": xT[b]["v"],
                "wq": (np.ascontiguousarray(Wq_w[:, cols]) * np.float32(0.125)).astype(np.float16),
                "wk": np.ascontiguousarray(Wk_w[:, cols]).astype(np.float16),
                "wv": np.ascontiguousarray(Wv_w[:, cols]).astype(np.float16),
                "bq": (Wq_b[cols] * np.float32(0.125)).reshape(2, 128).copy(),
                "bk": Wk_b[cols].reshape(2, 128).copy(),
                "bv": Wv_b[cols].reshape(1, D4).copy(),
                "fo": np.ascontiguousarray(fo_w[cols, :]).astype(np.float16),
            }
        )

    if _NC_CACHE is None:
        _NC_CACHE = build_nc()
    nc = _NC_CACHE

    res = bass_utils.run_bass_kernel_spmd(nc, in_maps, core_ids=list(range(NCORES)))

    attention = np.empty((B, NH, S, S), dtype=np.float32)
    output = np.empty((B, S, H), dtype=np.float32)
    acc = {b: np.zeros((H, S), dtype=np.float32) for b in range(B)}
    for core in range(NCORES):
        b, g = divmod(core, 4)
        r = res.results[core]
        at = r["attnT"]
        for i in range(HG):
            attention[b, g * HG + i] = at[i].T
        acc[b] += r["outT"].astype(np.float32)
    for b in range(B):
        output[b] = acc[b].T + fo_b[None, :]

    return output, attention
